# revision 34
# baseline (speedup 1.0000x reference)
"""Trainium2 Bass kernel for the DLI pairwise CE loss.

Reference computation:
  his   = encoder_output[b, his_turn_end_ids[b, t], :]          # [B, T, E] gather
  left  = his @ Wl.T ; right = his @ Wr.T                       # [B, T, 2]
  logits[b, j, k, :] = left[b, j] + right[b, k] + bias          # [B, T, T, 2]
  ce    = -log_softmax(logits)[label]   (label = (k == j-1))
  loss  = sum(ce[valid]) / count,  valid = (k < j) & (j < turn_len[b])

Two-class log-softmax depends only on d = z1 - z0:
  d[j,k] = dl[j] + dr[k] + db
  dl[j]  = his[j].(Wl[1]-Wl[0]),  dr[k] = his[k].(Wr[1]-Wr[0]),  db = b[1]-b[0]
  ce     = softplus(d) - d*label
With h = d/2 (the host halves the weight vectors) and m the valid mask:
  sum(m*softplus(d)) = sum(m*h) + sum(|m*h|)
                     + sum(log1p(exp(-2*|m*h|)) - ln2) + ln(2)*count
(|m*h| is 0 at masked positions, where log1p(exp(0)) - ln2 = 0 exactly, so
the shifted-log sum needs no mask; the host adds ln2*count), and the label term
sum_{label} d is separable into column sums of dl/2 and dr/2 with multiplier
-2.  All grid sums come for free from instruction accumulators
(scalar_tensor_tensor / activation accum_out), so the
device computes: gather -> two fused dot products -> PE transpose -> one
fused (bias+mask) op with row sums -> abs (row sums) -> Exp -> Ln (row sums)
per 128x128 group, with all per-partition partials collected in a [128, 10]
matrix that is DMA'd out; the host does the final 1280-value sum.

Sharding: data-parallel over batch; 4 batches per core on 8 cores. Each core
indirect-DMA-gathers its 4*64 turn-end rows (2 groups of 128 = partition dim)
from its bf16 [4*4096, 512] encoder slice and emits a [128, 10] f32 partial
matrix. Host sums the partials and divides by the valid-pair count.
"""

import numpy as np
import ml_dtypes

import concourse.bass as bass
import concourse.tile as tile
from concourse import bacc, mybir
from concourse.bass_utils import run_bass_kernel_spmd
from concourse.masks import make_identity

BSZ, SEQ, ENC, TMAX = 32, 4096, 512, 64
NCORES = 8
BPC = BSZ // NCORES   # batches per core = 4
GB = 128 // TMAX      # batches per 128-partition group = 2
NG = BPC // GB        # groups per core = 2
F32 = mybir.dt.float32
BF16 = mybir.dt.bfloat16
I32 = mybir.dt.int32
MW = NG * 128         # grid width = 256

_PROGRAM_CACHE: dict = {}


def _build_program(combo_eng: str = "sync", big_eng: str = "sync", out_eng: str = "sync"):
    nc = bacc.Bacc("TRN2", target_bir_lowering=False, debug=False)

    table = nc.dram_tensor("table", [BPC * SEQ, ENC], BF16, kind="ExternalInput")
    # combo cols: 0,1 = gather row ids (int32 bits), 2 = db/2,
    #             3+2g, 4+2g = label-correction multipliers for group g
    combo = nc.dram_tensor("combo", [128, 16], F32, kind="ExternalInput")
    # bigload cols: [0:256] = valid masks (both groups), [256:1280] = the two
    # halved 512-d weight-difference vectors, replicated across partitions
    bigload = nc.dram_tensor("bigload", [128, MW + 2 * ENC], BF16,
                             kind="ExternalInput")
    out = nc.dram_tensor("out", [128, 10], F32, kind="ExternalOutput")

    with tile.TileContext(nc) as tc:
        with (
            tc.tile_pool(name="const", bufs=1) as const,
            tc.tile_pool(name="work", bufs=2) as work,
            tc.tile_pool(name="psum", bufs=2, space="PSUM") as psum,
        ):
            # pre-load the one ACT table covering Abs+Exp+Ln so the compiler's
            # table-load pass doesn't insert a mid-kernel reload between Exp
            # and Ln (observed: 1.28us on the critical path)
            from concourse.hw_specs import get_activation_tables

            tid = list(get_activation_tables(nc.m.arch)).index(
                "natural_log_exp_and_others"
            )
            atl = mybir.InstLoadActFuncSet(
                name=f"I-{nc.next_id()}", act_func_set_id=tid, ins=[], outs=[]
            )
            nc.scalar.add_instruction(atl)

            # combo rides the gpsimd (SWDGE) queue so the gathers that
            # depend on it are same-engine next; bigload rides the otherwise
            # idle DVE HWDGE ring
            combo_t = const.tile([128, 16], F32)
            getattr(nc, combo_eng).dma_start(out=combo_t[:], in_=combo[:])
            big_t = const.tile([128, MW + 2 * ENC], BF16)
            getattr(nc, big_eng).dma_start(out=big_t[:], in_=bigload[:])
            mask_all = big_t[:, 0:MW]
            wc = big_t[:, MW : MW + 2 * ENC]
            # per-partition partial sums; cols: 0,1 = sum(mask*h) per group,
            # 2,3 = sum(mask*|h|) per group, 4,5 = sum(log1p - ln2) per
            # group, 6..9 = label corrections; host sums all 1280 values
            colmat = const.tile([128, 10], F32)
            Dm = const.tile([128, MW], F32)  # mask * d/2

            # issue both gathers back-to-back on gpsimd right behind combo
            his_tiles = []
            for g in range(NG):
                his = work.tile([128, ENC], BF16, tag=f"his_{g}")
                nc.gpsimd.indirect_dma_start(
                    out=his[:],
                    out_offset=None,
                    in_=table[:],
                    in_offset=bass.IndirectOffsetOnAxis(
                        ap=combo_t[:, g : g + 1].bitcast(I32), axis=0
                    ),
                )
                his_tiles.append(his)

            # identity (gpsimd work) only after the gathers are queued
            ident = const.tile([128, 128], F32)
            make_identity(nc, ident[:])
            halfc = const.tile([128, 1], F32)
            nc.vector.memset(halfc[:], 0.5)

            for g in range(NG):
                his = his_tiles[g]
                # fused dot products: dl[j] = sum(his[j]*wld)/... (weights are
                # pre-halved, so these are dl/2 and dr/2)
                scr_l = work.tile([128, ENC], F32, tag="scr_l")
                dl = work.tile([128, 1], F32, tag=f"dl_{g}")
                nc.vector.scalar_tensor_tensor(
                    out=scr_l[:],
                    in0=his[:],
                    scalar=1.0,
                    in1=wc[:, :ENC],
                    op0=mybir.AluOpType.mult,
                    op1=mybir.AluOpType.mult,
                    accum_out=dl[:],
                )
                scr_r = work.tile([128, ENC], F32, tag="scr_r")
                dr = work.tile([128, 1], F32, tag=f"dr_{g}")
                nc.vector.scalar_tensor_tensor(
                    out=scr_r[:],
                    in0=his[:],
                    scalar=1.0,
                    in1=wc[:, ENC:],
                    op0=mybir.AluOpType.mult,
                    op1=mybir.AluOpType.mult,
                    accum_out=dr[:],
                )
                dl2h = work.tile([128, 1], F32, tag=f"dl2h_{g}")
                nc.vector.tensor_add(dl2h[:], dl[:], combo_t[:, 2:3])
                # drB[p, k] = dr[k] on every partition
                drB = psum.tile([128, 128], F32)
                nc.tensor.transpose(
                    out=drB[:],
                    in_=dr[:].to_broadcast([128, 128]),
                    identity=ident[:],
                )
                # Dm = (dr[k] + dl2h[j]) * mask = mask * d/2, and its row sums
                nc.vector.scalar_tensor_tensor(
                    out=Dm[:, g * 128 : (g + 1) * 128],
                    in0=drB[:],
                    scalar=dl2h[:],
                    in1=mask_all[:, g * 128 : (g + 1) * 128],
                    op0=mybir.AluOpType.add,
                    op1=mybir.AluOpType.mult,
                    accum_out=colmat[:, g : g + 1],
                )
                # label corrections: -2*sum(dl2h[label j]), -2*sum(drh[label k])
                nc.vector.tensor_mul(
                    colmat[:, 6 + 2 * g : 7 + 2 * g],
                    dl2h[:],
                    combo_t[:, 3 + 2 * g : 4 + 2 * g],
                )
                nc.vector.tensor_mul(
                    colmat[:, 7 + 2 * g : 8 + 2 * g],
                    dr[:],
                    combo_t[:, 4 + 2 * g : 5 + 2 * g],
                )
                # |mask*h| on DVE via (x*-1) max x, with free row sums;
                # per-group so group 0's tail overlaps group 1's projections
                absh = work.tile([128, 128], F32, tag=f"absh_{g}")
                nc.vector.scalar_tensor_tensor(
                    out=absh[:],
                    in0=Dm[:, g * 128 : (g + 1) * 128],
                    scalar=-1.0,
                    in1=Dm[:, g * 128 : (g + 1) * 128],
                    op0=mybir.AluOpType.mult,
                    op1=mybir.AluOpType.max,
                    accum_out=colmat[:, 2 + g : 3 + g],
                )
                # masked positions have |mask*h| = 0 -> expd = 1, which the
                # shifted Ln below maps to exactly 0 (no re-mask needed)
                expd = work.tile([128, 128], F32, tag=f"expd_{g}")
                nc.scalar.activation(
                    out=expd[:],
                    in_=absh[:],
                    func=mybir.ActivationFunctionType.Exp,
                    scale=-2.0,
                )
                # ln((1+expd)/2) = log1p(expd) - ln2: masked positions
                # (expd = 1) contribute exactly 0; the host adds ln2*count back
                lnd = work.tile([128, 128], F32, tag=f"lnd_{g}")
                nc.scalar.activation(
                    out=lnd[:],
                    in_=expd[:],
                    func=mybir.ActivationFunctionType.Ln,
                    bias=halfc[:],
                    scale=0.5,
                    accum_out=colmat[:, 4 + g : 5 + g],
                )

            getattr(nc, out_eng).dma_start(out=out[:], in_=colmat[:])

    nc.compile()
    return nc


def _get_program():
    if "nc" not in _PROGRAM_CACHE:
        _PROGRAM_CACHE["nc"] = _build_program()
    return _PROGRAM_CACHE["nc"]


def _make_in_maps(enc_bf16, ids, tl, W_, b_):
    # halved difference vectors, bf16, replicated across partitions
    wld = (W_[1, :ENC] - W_[0, :ENC]) * 0.5
    wrd = (W_[1, ENC:] - W_[0, ENC:]) * 0.5
    dbh = float(b_[1] - b_[0]) * 0.5
    wcomb = np.concatenate([wld, wrd]).astype(ml_dtypes.bfloat16)

    j = np.arange(TMAX)
    in_maps = []
    for c in range(NCORES):
        table = enc_bf16[c * BPC : (c + 1) * BPC].reshape(BPC * SEQ, ENC)
        combo = np.zeros((128, 16), np.float32)
        combo[:, 2] = dbh
        bigload = np.zeros((128, MW + 2 * ENC), ml_dtypes.bfloat16)
        bigload[:, MW:] = wcomb[None, :]
        idx = np.zeros((128, NG), np.int32)
        for g in range(NG):
            for lb2 in range(GB):
                lb = g * GB + lb2
                bi = c * BPC + lb
                L = int(tl[bi])
                rows = slice(lb2 * TMAX, (lb2 + 1) * TMAX)
                idx[rows, g] = (lb * SEQ + ids[bi]).astype(np.int32)
                mrow = (j[:, None] > j[None, :]) & (j[:, None] < L)
                c0 = g * 128 + lb2 * TMAX
                bigload[rows, c0 : c0 + TMAX] = mrow.astype(ml_dtypes.bfloat16)
                # label pairs (j, j-1), 1<=j<L: subtract d = 2*(dl2h + drh)
                combo[rows, 3 + 2 * g] = np.where((j >= 1) & (j < L), -2.0, 0.0)
                combo[rows, 4 + 2 * g] = np.where(j < L - 1, -2.0, 0.0)
        combo[:, 0:NG] = idx.view(np.float32)
        in_maps.append({"table": table, "combo": combo, "bigload": bigload})
    return in_maps


def kernel(encoder_output, his_turn_end_ids, turn_lengths, W, b):
    enc = np.asarray(encoder_output, dtype=np.float32)
    enc_bf16 = np.ascontiguousarray(enc.astype(ml_dtypes.bfloat16))
    ids = np.asarray(his_turn_end_ids).astype(np.int64)
    tl = np.asarray(turn_lengths).astype(np.int64)
    W_ = np.asarray(W, dtype=np.float32)
    b_ = np.asarray(b, dtype=np.float32)

    in_maps = _make_in_maps(enc_bf16, ids, tl, W_, b_)
    nc = _get_program()
    res = run_bass_kernel_spmd(nc, in_maps, list(range(NCORES)))
    total = sum(float(np.asarray(r["out"], dtype=np.float64).sum()) for r in res.results)

    count = int(sum(int(L) * (int(L) - 1) // 2 for L in tl.tolist()))
    # the device Ln columns hold log1p(exp(-|d|)) - ln2 per valid pair
    total += float(np.log(2.0)) * count
    count = max(count, 1)
    return np.float32(total / count)
